# revision 40
# baseline (speedup 1.0000x reference)
"""Trainium2 Bass kernel for CrossLayerSharedZOlmoeSparseMoeBlock.

Strategy (expert-parallel, 8 cores):
  host: full routing math in fp32 numpy (predictor MLP + gumbel argmax +
        router softmax + top-8-of-16) -> per-expert token index lists;
        expert token lists split into panels (one expert per panel) and
        packed into an 8-core x npanels grid whose shared compile-time
        panel-size vector is optimized at runtime (sum ~= 2080 tokens vs
        2048 ideal); token buffers gathered/compacted per panel in bf16.
  device (one kernel launch): per core, per panel: gate/up matmuls in
        bf16 (fp32 PSUM), silu*up*routing-weight fused at PSUM eviction
        (scalar silu -> vector mult -> gpsimd mult by broadcast weight
        row); down-proj with tokens as the moving dim (stationary = Wd
        128x128 tiles) so there is no 128-chunk quantization; evictions
        stage a whole [2048H x slice] block in SBUF and stream out as
        ~MB-sized DMAs (small descriptors starve behind multi-MB
        prefetch descriptors on the 16 shared DMA engines).  Startup
        splits the critical first bytes across the sync/scalar/gpsimd
        rings in consumption order; wd and next-panel tokens are paced
        into the gpsimd eviction stream mid-panel.
  host: scatter-add compact fp32 outputs into y.

bf16 matmuls run at ~0.425ns/row on the PE; per-core PE floor is
384 cycles/token.  Aggregate rel err ~4e-3 (tolerance 2e-2).
"""
import contextlib
import ctypes
import math
import os
import random
import sys
import types

import ml_dtypes
import numpy as np

sys.path.insert(0, "/opt/trn_rl_repo")

# ---------------------------------------------------------------------------
# NTFF profile hook shim (antenv.axon_hooks is absent in this image; bass's
# trace=True path imports it). Lets us read HW exec time via neuron profile.
# ---------------------------------------------------------------------------
_SO_PATH = "/opt/axon/libaxon_pjrt.so"


def _ntff_profile_via_ctypes(so_path):
    try:
        lib = ctypes.CDLL(so_path)
    except OSError:
        return None
    if not hasattr(lib, "axon_start_nrt_profile"):
        return None
    lib.axon_start_nrt_profile.argtypes = [ctypes.POINTER(ctypes.c_int64), ctypes.c_size_t]
    lib.axon_start_nrt_profile.restype = ctypes.c_int64
    lib.axon_stop_nrt_profile.argtypes = [ctypes.c_char_p]
    lib.axon_stop_nrt_profile.restype = ctypes.c_int64

    @contextlib.contextmanager
    def _hook(output_dir, device_ids):
        import jax

        jax.devices()
        if device_ids:
            ids = (ctypes.c_int64 * len(device_ids))(*device_ids)
            rc = lib.axon_start_nrt_profile(ids, len(device_ids))
        else:
            rc = lib.axon_start_nrt_profile(None, 0)
        if rc != 0:
            raise RuntimeError(f"axon_start_nrt_profile rc={rc}")
        try:
            yield
        finally:
            n = lib.axon_stop_nrt_profile(str(output_dir).encode())
            print(f"ntff profile: {n} file(s) -> {output_dir}", file=sys.stderr)

    return _hook


def _install_hook():
    if "antenv.axon_hooks" in sys.modules:
        return
    mod = types.ModuleType("antenv.axon_hooks")
    _h = [_ntff_profile_via_ctypes(_SO_PATH)]
    mod.get_axon_ntff_profile_hook = lambda: _h[0]
    mod.set_axon_ntff_profile_hook = lambda h: _h.__setitem__(0, h)
    sys.modules["antenv.axon_hooks"] = mod
    try:
        import antenv

        antenv.axon_hooks = mod
    except ImportError:
        pass


_install_hook()

import concourse.mybir as mybir  # noqa: E402
import concourse.tile as tile  # noqa: E402
from concourse import bacc  # noqa: E402
from concourse.bass_utils import run_bass_kernel_spmd  # noqa: E402

F32 = mybir.dt.float32
BF16 = mybir.dt.bfloat16
ALU = mybir.AluOpType
ACTF = mybir.ActivationFunctionType

# problem shapes (hardcoded per contest rules)
B, S, H = 1, 2048, 2048
T = B * S
E, F = 16, 1024
Z, M = 8, 512
TOP_K = 8
EPS = 1e-10
TAU = 1.0
N_CORES = 8
P = 128
KH = H // P          # 16 contraction chunks over H
MF = F // P          # 8 F tiles for gate/up (also down contraction chunks)
KF = F // P
HT = H // P          # 16 output H tiles for down-proj

TRACE = bool(int(os.environ.get("BASSMOE_TRACE", "0")))
BF = ml_dtypes.bfloat16

_timings = {}
_build_cache = {}


def _slices(C, progressive):
    """Token sub-slices for a panel: each <= 512 (one PSUM bank).

    progressive (panel 0): small first slice so compute starts before the
    whole token buffer lands. Otherwise near-equal chunks (wide matmuls --
    narrow moving dims expose the PE's per-instruction cost).
    """
    out = []
    off = 0
    if progressive and C > 128:
        out.append((0, 128))
        off = 128
    rem = C - off
    if rem > 0:
        n = (rem + 511) // 512
        base, ex = divmod(rem, n)
        for i in range(n):
            w = base + (1 if i < ex else 0)
            out.append((off, w))
            off += w
    return out


def _mm_exposure_ns(w):
    """Measured per-matmul exposed overhead vs moving width (ns)."""
    if w >= 470:
        return 3.0
    if w >= 300:
        return 5.0
    if w >= 230:
        return 8.0
    if w >= 180:
        return 25.0
    return 45.0


def _panel_tax_tokens(s):
    """Extra PE time of a panel beyond 384 cycles/token, in token-equivs."""
    tax_ns = 0.0
    ns = 0
    for (_, w) in _slices(s, False):
        tax_ns += 384 * _mm_exposure_ns(w)    # 384 matmuls per slice
        ns += 1
    return tax_ns / 160.0 + 8.0 * ns + 6.0    # +fixed per-slice/panel cost


# ---------------------------------------------------------------------------
# K2: expert kernel. sizes = per-panel token counts (compile-time, shared
# across cores, ascending).
# ---------------------------------------------------------------------------
def build_k2(sizes):
    nc = bacc.Bacc(None, target_bir_lowering=False)
    ins, outs = [], []
    for j, Sz in enumerate(sizes):
        ins.append((
            nc.dram_tensor(f"xg{j}", [P, KH * Sz], BF16, kind="ExternalInput"),
            nc.dram_tensor(f"wg{j}", [MF, P, KH * P], BF16, kind="ExternalInput"),
            nc.dram_tensor(f"wu{j}", [MF, P, KH * P], BF16, kind="ExternalInput"),
            nc.dram_tensor(f"wd{j}", [P, KF * HT * P], BF16, kind="ExternalInput"),
            nc.dram_tensor(f"wv{j}", [P, Sz], F32, kind="ExternalInput"),
        ))
        outs.append(nc.dram_tensor(f"out{j}", [P, HT, Sz], BF16,
                                   kind="ExternalOutput"))

    NP = len(sizes)
    SL = max(cw for j, Sz in enumerate(sizes)
             for (_, cw) in _slices(Sz, j == 0))
    with tile.TileContext(nc) as tc:
        with tc.tile_pool(name="xg", bufs=2) as xg_pool, \
             tc.tile_pool(name="wgu", bufs=8) as wgu_pool, \
             tc.tile_pool(name="wd", bufs=1) as wd_pool, \
             tc.tile_pool(name="wvp", bufs=2) as wv_pool, \
             tc.tile_pool(name="act", bufs=2) as act_pool, \
             tc.tile_pool(name="sgp", bufs=3) as sg_pool, \
             tc.tile_pool(name="a1p", bufs=3) as a1_pool, \
             tc.tile_pool(name="ev", bufs=2) as ev_pool, \
             tc.tile_pool(name="warm", bufs=1) as warm_pool, \
             tc.tile_pool(name="psg", bufs=2, space="PSUM") as psg, \
             tc.tile_pool(name="psu", bufs=2, space="PSUM") as psu, \
             tc.tile_pool(name="psd", bufs=4, space="PSUM") as psd:

            # PE warmup (HAM unthrottle) while the first DMAs land.
            warm = warm_pool.tile([P, 512], BF16, name="warm")
            nc.vector.memset(warm[:, :], 0.0)
            for i in range(6):
                wps = (psg if i % 2 == 0 else psu).tile(
                    [P, 512], F32, name=("pg" if i % 2 == 0 else "pu"))
                nc.tensor.matmul(out=wps[:], lhsT=warm[:, :P],
                                 rhs=warm[:, :512], start=True, stop=True)

            xg_t = [None] * NP
            wv_t = [None] * NP
            wgu_t = {}           # (j, m) -> (wg_tile, wu_tile)

            def load_wgu(j, m, q):
                wgt = wgu_pool.tile([P, KH * P], BF16, name="wg")
                q.dma_start(out=wgt[:], in_=ins[j][1][m])
                wut = wgu_pool.tile([P, KH * P], BF16, name="wu")
                q.dma_start(out=wut[:], in_=ins[j][2][m])
                wgu_t[(j, m)] = (wgt, wut)

            # Panel-0 startup: rings are cold for ~10us; split the critical
            # bytes across rings in consumption order: wg[0] + slice
            # k-halves on sync, xg slice 0 + wu[0] + other k-halves on
            # scalar, wv0 + pairs 1-2 on gpsimd, pair 3 on sync.
            xg_t[0] = xg_pool.tile([P, KH * sizes[0]], BF16, name="xg")
            CS0 = _slices(sizes[0], True)
            wg00 = wgu_pool.tile([P, KH * P], BF16, name="wg")
            nc.sync.dma_start(out=wg00[:], in_=ins[0][1][0])
            c0, cw = CS0[0]
            nc.scalar.dma_start(out=xg_t[0][:, :KH * cw],
                                in_=ins[0][0][:, :KH * cw])
            wu00 = wgu_pool.tile([P, KH * P], BF16, name="wu")
            nc.scalar.dma_start(out=wu00[:], in_=ins[0][2][0])
            wgu_t[(0, 0)] = (wg00, wu00)
            for (c0, cw) in CS0[1:]:
                o, half = KH * c0, KH // 2 * cw
                nc.sync.dma_start(out=xg_t[0][:, o:o + half],
                                  in_=ins[0][0][:, o:o + half])
                nc.scalar.dma_start(out=xg_t[0][:, o + half:o + KH * cw],
                                    in_=ins[0][0][:, o + half:o + KH * cw])
            wv_t[0] = wv_pool.tile([P, sizes[0]], F32, name="wv")
            nc.gpsimd.dma_start(out=wv_t[0][:], in_=ins[0][4][:])
            load_wgu(0, 1, nc.gpsimd)
            load_wgu(0, 2, nc.gpsimd)
            load_wgu(0, 3, nc.sync)

            for j, Sz in enumerate(sizes):
                CS = _slices(Sz, j == 0)
                nxt_xg = []
                if j + 1 < NP:
                    xg_t[j + 1] = xg_pool.tile([P, KH * sizes[j + 1]], BF16,
                                               name="xg")
                    nxt_xg = [(KH * c0, KH * cw)
                              for (c0, cw) in _slices(sizes[j + 1], False)]
                wd_tile = None
                act = act_pool.tile([P, KF, Sz], BF16, name="act")

                for m in range(MF):
                    wg_tile, wu_tile = wgu_t.pop((j, m))
                    # prefetch weights 2 m-tiles ahead (scalar ring)
                    mm2 = m + 2
                    if mm2 < MF:
                        if (j, mm2) not in wgu_t:
                            load_wgu(j, mm2, nc.scalar)
                    elif j + 1 < NP:
                        load_wgu(j + 1, mm2 - MF, nc.scalar)
                    for si, (c0, cw) in enumerate(CS):
                        o = KH * c0
                        pg = psg.tile([P, 512], F32, name="pg")[:, :cw]
                        pu = psu.tile([P, 512], F32, name="pu")[:, :cw]
                        for k in range(KH):
                            nc.tensor.matmul(
                                out=pg[:], lhsT=wg_tile[:, k * P:(k + 1) * P],
                                rhs=xg_t[j][:, o + k * cw:o + (k + 1) * cw],
                                start=(k == 0), stop=(k == KH - 1))
                        for k in range(KH):
                            nc.tensor.matmul(
                                out=pu[:], lhsT=wu_tile[:, k * P:(k + 1) * P],
                                rhs=xg_t[j][:, o + k * cw:o + (k + 1) * cw],
                                start=(k == 0), stop=(k == KH - 1))
                        sg = sg_pool.tile([P, SL], F32, name="sg")[:, :cw]
                        nc.scalar.activation(out=sg[:], in_=pg[:],
                                             func=ACTF.Silu, bias=0.0,
                                             scale=1.0)
                        a1 = a1_pool.tile([P, SL], F32, name="a1")[:, :cw]
                        nc.vector.tensor_tensor(out=a1[:], in0=sg[:],
                                                in1=pu[:], op=ALU.mult)
                        nc.gpsimd.tensor_tensor(
                            out=act[:, m, c0:c0 + cw], in0=a1[:],
                            in1=wv_t[j][:, c0:c0 + cw], op=ALU.mult)
                    # paced gpsimd-ring loads (execute after m's evictions):
                    # wd{j} (4MB) only after m=4 so it cannot starve the gu
                    # weight stream; next panel's tokens after m=5.
                    if m == 4:
                        wd_tile = wd_pool.tile([P, KF * HT * P], BF16,
                                               name="wd")
                        nc.gpsimd.dma_start(out=wd_tile[:], in_=ins[j][3][:])
                    elif m >= 5 and nxt_xg:
                        o, ln = nxt_xg.pop(0)
                        nc.gpsimd.dma_start(out=xg_t[j + 1][:, o:o + ln],
                                            in_=ins[j + 1][0][:, o:o + ln])
                while nxt_xg:
                    o, ln = nxt_xg.pop(0)
                    nc.gpsimd.dma_start(out=xg_t[j + 1][:, o:o + ln],
                                        in_=ins[j + 1][0][:, o:o + ln])
                if j + 1 < NP:
                    wv_t[j + 1] = wv_pool.tile([P, sizes[j + 1]], F32,
                                               name="wv")
                    nc.gpsimd.dma_start(out=wv_t[j + 1][:],
                                        in_=ins[j + 1][4][:])

                # down projection: tokens moving, Wd 128x128 stationary.
                # Evictions (vector/scalar alternating) fill a slice-wide
                # staging buffer; ONE big output DMA per slice (the final
                # slice drains in 4-H-tile chunks so the tail overlaps).
                for si, (c0, cw) in enumerate(CS):
                    last_slice = (j == NP - 1 and si == len(CS) - 1)
                    ev = ev_pool.tile([P, HT, SL], BF16, name="ev")
                    for ht in range(HT):
                        pd = psd.tile([P, 512], F32, name="pd")[:, :cw]
                        for k in range(KF):
                            wo = (k * HT + ht) * P
                            nc.tensor.matmul(
                                out=pd[:],
                                lhsT=wd_tile[:, wo:wo + P],
                                rhs=act[:, k, c0:c0 + cw],
                                start=(k == 0), stop=(k == KF - 1))
                        if ht % 2 == 0:
                            nc.vector.tensor_scalar(
                                out=ev[:, ht, :cw], in0=pd[:],
                                scalar1=1.0, scalar2=None, op0=ALU.mult)
                        else:
                            nc.scalar.activation(
                                out=ev[:, ht, :cw], in_=pd[:],
                                func=ACTF.Copy, bias=0.0, scale=1.0)
                        if last_slice and ht % 2 == 1:
                            # rings are empty at the end of the kernel, so
                            # small chunks retire promptly; alternate rings
                            # so the final drain overlaps the last evictions
                            oq = nc.sync if (ht // 2) % 2 == 0 else nc.gpsimd
                            oq.dma_start(
                                out=outs[j][:, ht - 1:ht + 1, c0:c0 + cw],
                                in_=ev[:, ht - 1:ht + 1, :cw])
                    if not last_slice:
                        nc.sync.dma_start(out=outs[j][:, :, c0:c0 + cw],
                                          in_=ev[:, :, :cw])
    nc.compile()
    return nc


# ---------------------------------------------------------------------------
# host routing (exact fp32 replication of the reference)
# ---------------------------------------------------------------------------
def _host_routing(x, gumbel_u, W1, b1, W2, b2, gate_w, U, alpha):
    h1 = x @ W1.T + b1
    h1 *= 1.0 / (1.0 + np.exp(-h1))                       # silu
    zl = h1 @ W2.T + b2
    g = -np.log(-np.log(gumbel_u + EPS) + EPS)
    s = (zl + g) / TAU
    s -= s.max(-1, keepdims=True)
    es = np.exp(s)
    soft = es / es.sum(-1, keepdims=True)
    hard = np.zeros_like(soft)
    hard[np.arange(T), soft.argmax(-1)] = 1.0
    z = (hard + soft) - soft                              # straight-through
    rl = x @ gate_w.T + np.float32(alpha) * (z @ U)
    rl -= rl.max(-1, keepdims=True)
    er = np.exp(rl)
    rw = er / er.sum(-1, keepdims=True)
    order = np.argsort(-rw, axis=1, kind="stable")[:, :TOP_K]
    topw = np.take_along_axis(rw, order, axis=1)
    return order, topw


# ---------------------------------------------------------------------------
# panel packing: choose a shared panel-size vector S (ascending) and an
# assignment of single-expert token pieces to the 8 x len(S) panel grid.
# ---------------------------------------------------------------------------
def _greedy_fill(S, loads):
    """S descending. Returns pieces [(expert, off, ln, class_idx)] or None."""
    avail = []
    for j, s in enumerate(S):
        avail += [[s, j] for _ in range(N_CORES)]
    avail.sort(key=lambda t: -t[0])
    pieces = []
    for e in sorted(range(len(loads)), key=lambda i: -loads[i]):
        rem = int(loads[e])
        off = 0
        while rem > 0:
            if avail and rem >= avail[0][0]:
                cap, j = avail.pop(0)
                pieces.append((e, off, cap, j))
                off += cap
                rem -= cap
            else:
                k = None
                for i in range(len(avail) - 1, -1, -1):
                    if avail[i][0] >= rem:
                        k = i
                        break
                if k is None:
                    return None
                cap, j = avail.pop(k)
                pieces.append((e, off, rem, j))
                rem = 0
    return pieces


def _pack_cost(Sd):
    # smallest class runs FIRST (ascending panel order); if it is short,
    # its gate/up phase cannot hide the 4MB wd load + weight pairs on the
    # still-ramping DMA rings -> penalize first panels under ~480 tokens
    startup_pen = max(0, 480 - min(Sd))
    return sum(s + _panel_tax_tokens(s) for s in Sd) + startup_pen


def _pack(loads):
    halves = sorted([l // 2 for l in loads] + [l - l // 2 for l in loads],
                    reverse=True)
    cands = [[halves[0], halves[8], halves[16], halves[24]]]
    best = None
    for Sd in cands:
        Sd = sorted(Sd, reverse=True)
        if _greedy_fill(Sd, loads) is not None:
            c = _pack_cost(Sd)
            if best is None or c < best[0]:
                best = (c, Sd)
    rnd = random.Random(12345)
    for _ in range(30000):
        m = rnd.choice([3, 4, 4, 4, 5])
        Sd = sorted((rnd.randrange(380, 769) for _ in range(m)),
                    reverse=True)
        c = _pack_cost(Sd)
        if best is not None and c >= best[0]:
            continue
        if sum(Sd) * N_CORES < sum(loads):
            continue
        if _greedy_fill(Sd, loads) is not None:
            best = (c, Sd)
    Sd = best[1]
    pieces = _greedy_fill(Sd, loads)
    # relabel classes so panel sizes run ascending (small panel first)
    orderc = sorted(range(len(Sd)), key=lambda j: Sd[j])
    remap = {old: new for new, old in enumerate(orderc)}
    sizes = [Sd[j] for j in orderc]
    grid = {}
    used = [0] * len(sizes)
    for (e, off, ln, jold) in sorted(pieces, key=lambda p: -p[2]):
        jn = remap[jold]
        c = used[jn]
        used[jn] += 1
        grid[(c, jn)] = (e, off, ln)
    return sizes, grid


def kernel(hidden_states, gumbel_u, W1, b1, W2, b2, gate_w, U, alpha, Wg, Wu, Wd):
    import time as _time

    t_start = _time.time()
    x = np.ascontiguousarray(np.asarray(hidden_states, np.float32).reshape(T, H))

    # ---- routing on host ----
    t0 = _time.time()
    order, topw = _host_routing(
        x, np.asarray(gumbel_u, np.float32),
        np.asarray(W1, np.float32), np.asarray(b1, np.float32),
        np.asarray(W2, np.float32), np.asarray(b2, np.float32),
        np.asarray(gate_w, np.float32), np.asarray(U, np.float32), alpha)
    idxs = [None] * E
    wvals = [None] * E
    for e in range(E):
        rows, cols = np.nonzero(order == e)
        idxs[e] = rows
        wvals[e] = topw[rows, cols].astype(np.float32)
    loads = [len(idxs[e]) for e in range(E)]
    _timings["routing"] = _time.time() - t0

    # ---- pack pieces into 8 cores x npanels ----
    t0 = _time.time()
    sizes, grid = _pack(loads)
    NP = len(sizes)
    _timings["pack"] = _time.time() - t0
    _timings["sizes"] = tuple(sizes)

    # ---- weight/activation prep (bf16, transposed+interleaved) ----
    t0 = _time.time()
    xT = np.ascontiguousarray(
        x.reshape(T, KH, P).transpose(2, 1, 0).astype(BF))   # [128, 16, T]
    WgB = np.asarray(Wg, np.float32).astype(BF)
    WuB = np.asarray(Wu, np.float32).astype(BF)
    WdB = np.asarray(Wd, np.float32).astype(BF)
    # wgt[e,m,p,k,j] = Wg[e, m*128+j, k*128+p]
    WgT = np.ascontiguousarray(
        WgB.reshape(E, MF, P, KH, P).transpose(0, 1, 4, 3, 2))
    WuT = np.ascontiguousarray(
        WuB.reshape(E, MF, P, KH, P).transpose(0, 1, 4, 3, 2))
    # wdt[e,p,k,ht,j] = Wd[e, ht*128+j, k*128+p]
    WdT = np.ascontiguousarray(
        WdB.reshape(E, HT, P, KF, P).transpose(0, 4, 3, 1, 2))

    in_maps = []
    for c in range(N_CORES):
        mdict = {}
        for j in range(NP):
            Sz = sizes[j]
            e, off, ln = grid.get((c, j), (0, 0, 0))
            xg3 = np.zeros((P, KH, Sz), BF)
            wvp = np.zeros((Sz,), np.float32)
            if ln > 0:
                sel = idxs[e][off:off + ln]
                xg3[:, :, :ln] = xT[:, :, sel]
                wvp[:ln] = wvals[e][off:off + ln]
                mdict[f"wg{j}"] = WgT[e].reshape(MF, P, KH * P)
                mdict[f"wu{j}"] = WuT[e].reshape(MF, P, KH * P)
                mdict[f"wd{j}"] = WdT[e].reshape(P, KF * HT * P)
            else:
                mdict[f"wg{j}"] = np.zeros((MF, P, KH * P), BF)
                mdict[f"wu{j}"] = np.zeros((MF, P, KH * P), BF)
                mdict[f"wd{j}"] = np.zeros((P, KF * HT * P), BF)
            # slice-major flat pack (must match kernel's per-slice offsets)
            xg = np.concatenate(
                [np.ascontiguousarray(xg3[:, :, c0:c0 + cw]).reshape(P, KH * cw)
                 for (c0, cw) in _slices(Sz, j == 0)], axis=1)
            mdict[f"xg{j}"] = xg
            mdict[f"wv{j}"] = np.ascontiguousarray(
                np.broadcast_to(wvp, (P, Sz)))
        in_maps.append(mdict)
    _timings["dispatch"] = _time.time() - t0

    t0 = _time.time()
    key = tuple(sizes)
    nc2 = _build_cache.get(key)
    if nc2 is None:
        nc2 = build_k2(sizes)
        _build_cache[key] = nc2
    _timings["k2_build"] = _time.time() - t0

    t0 = _time.time()
    res2 = run_bass_kernel_spmd(nc2, in_maps, list(range(N_CORES)), trace=TRACE)
    _timings["k2_run"] = _time.time() - t0
    if TRACE:
        _timings["k2_hw_ns"] = res2.exec_time_ns

    # ---- host combine (unshard) ----
    t0 = _time.time()
    y = np.zeros((T, H), np.float32)
    for c in range(N_CORES):
        for j in range(NP):
            e, off, ln = grid.get((c, j), (0, 0, 0))
            if ln == 0:
                continue
            oc = res2.results[c][f"out{j}"]              # [128, 16, Sz] bf16
            # element (p, ht, c) -> y[token c, ht*128 + p]
            y[idxs[e][off:off + ln]] += (
                oc[:, :, :ln].transpose(2, 1, 0).reshape(ln, H).astype(np.float32))
    _timings["combine"] = _time.time() - t0
    _timings["total"] = _time.time() - t_start
    return y.reshape(B, S, H)
